# revision 4
# baseline (speedup 1.0000x reference)
import os
import sys

import numpy as np

sys.path.insert(0, "/opt/trn_rl_repo")

import concourse.bacc as bacc
import concourse.bass as bass
import concourse.mybir as mybir
from concourse import masks
from concourse.bass_utils import run_bass_kernel_spmd
from concourse.tile import TileContext

B, DIM, H, HKV, D = 2, 4096, 32, 8, 128
R = H // HKV
PAGE, WINDOW, TOPK = 16, 4096, 4096
START = 32768
PREF = START - WINDOW          # 28672 prefix tokens
NCH = PREF // (128 * PAGE)     # 14 chunks of 2048 tokens (128 pages x 16)
NP = PREF // PAGE              # 1792 pages
T = TOPK // PAGE               # 256 pages selected
SUF = WINDOW                   # 4096 suffix tokens from cache
NSC = SUF // 128               # 32 suffix chunks of 128 tokens
SCALE = 1.0 / float(np.sqrt(D))
BISECT_ITERS = 30

F32 = mybir.dt.float32
X = mybir.AxisListType.X
OP = mybir.AluOpType


def build_nc():
    nc = bacc.Bacc()
    ck = nc.declare_dram_parameter("ck", [B, START, D], F32, isOutput=False)
    cv = nc.declare_dram_parameter("cv", [B, START, D], F32, isOutput=False)
    qT = nc.declare_dram_parameter("qT", [D, B * R], F32, isOutput=False)
    out = nc.declare_dram_parameter("out", [B, 2, R, 132], F32, isOutput=True)

    from contextlib import ExitStack

    with TileContext(nc) as tc, ExitStack() as es:
        cpool = es.enter_context(tc.tile_pool(name="consts", bufs=1))
        ident = cpool.tile([128, 128], F32)
        masks.make_identity(nc, ident[:])
        ones = cpool.tile([128, 1], F32)
        nc.vector.memset(ones[:], 1.0)
        qsb = cpool.tile([128, B * R], F32)
        nc.sync.dma_start(out=qsb[:], in_=qT[:, :])
        ones_row = cpool.tile([1, 128], F32)
        nc.vector.memset(ones_row[:], 1.0)

        def bcast_rows(dst_sb, src_1xn, n):
            # replicate [1, n] across 128 partitions via PE outer product
            bc_ps = pp_ms.tile([128, 128], F32, tag="ms", name="bc_ps")
            nc.tensor.matmul(bc_ps[:, :n], ones_row[:], src_1xn,
                             start=True, stop=True)
            nc.vector.tensor_copy(dst_sb, bc_ps[:, :n])

        spool = es.enter_context(tc.tile_pool(name="state", bufs=1))
        # per-token scores, page-swizzled: [part=page%128, (chunk, within, head)]
        sc = [spool.tile([128, NCH, PAGE, R], F32, name=f"sc{i}") for i in range(B)]
        pmaxT = spool.tile([128, NCH, B, R], F32)   # per-page max scores
        pm01 = spool.tile([128, NCH, B, R], F32)    # selection mask
        ge01 = spool.tile([128, NCH, B, R], F32)
        gmax8 = spool.tile([8, 1], F32)
        gmaxf = spool.tile([1, 8], F32)
        lo = spool.tile([1, 8], F32)
        hi = spool.tile([1, 8], F32)
        mid = spool.tile([1, 8], F32)
        cnt = spool.tile([1, 8], F32)
        sel = spool.tile([1, 8], F32)
        nsel = spool.tile([1, 8], F32)
        bt1 = spool.tile([1, 8], F32)
        gmaxm = [spool.tile([R, 1], F32, name=f"gmaxm{i}") for i in range(B)]
        bt2 = spool.tile([1, 8], F32)
        tmp128 = spool.tile([8, 128], F32)
        ssc = [spool.tile([128, NSC, R], F32, name=f"ssc{i}") for i in range(B)]
        msuf = [spool.tile([R, 1], F32, name=f"msuf{i}") for i in range(B)]
        msuf_f = [spool.tile([1, R], F32, name=f"msuff{i}") for i in range(B)]
        msufb = [spool.tile([128, R], F32, name=f"msufb{i}") for i in range(B)]
        midb = spool.tile([128, 8], F32)
        gmaxb = spool.tile([128, 8], F32)

        kpool = es.enter_context(tc.tile_pool(name="k", bufs=2))
        ktpool = es.enter_context(tc.tile_pool(name="kt", bufs=6))
        vpool = es.enter_context(tc.tile_pool(name="v", bufs=3))
        stgpool = es.enter_context(tc.tile_pool(name="stg", bufs=2))

        pp_kt = es.enter_context(tc.tile_pool(name="pp_kt", bufs=3, space="PSUM"))
        pp_qk = es.enter_context(tc.tile_pool(name="pp_qk", bufs=2, space="PSUM"))
        pp_av = es.enter_context(tc.tile_pool(name="pp_av", bufs=1, space="PSUM"))
        pp_ms = es.enter_context(tc.tile_pool(name="pp_ms", bufs=1, space="PSUM"))

        warm_ps = pp_kt.tile([128, 128], F32, tag="kt", name="warm_ps")
        nc.tensor.transpose(warm_ps[:], ident[:], ident[:])

        def qk_block(ksrc_ap, qk_ps, j):
            # ksrc_ap: [128 tok, 128 d] natural -> scores [128 tok, R] in psum cols
            kt_ps = pp_kt.tile([128, 128], F32, tag="kt")
            nc.tensor.transpose(kt_ps[:], ksrc_ap, ident[:])
            kt_sb = ktpool.tile([128, 128], F32, tag="kt_sb")
            nc.vector.tensor_copy(kt_sb[:], kt_ps[:])
            nc.tensor.matmul(
                qk_ps[:, j * R:(j + 1) * R], kt_sb[:], qrhs,
                start=True, stop=True,
            )

        # ---------------- prefix QK + page max ----------------
        for b in range(B):
            qrhs = qsb[:, b * R:(b + 1) * R]
            for c in range(NCH):
                ksb = kpool.tile([128, PAGE, 128], F32, tag="ksb")
                nc.sync.dma_start(
                    out=ksb[:],
                    in_=ck[b, c * 2048:(c + 1) * 2048, :].rearrange(
                        "(p w) d -> p w d", p=128
                    ),
                )
                qk_ps = pp_qk.tile([128, PAGE * R], F32, tag="qk")
                for w in range(PAGE):
                    qk_block(ksb[:, w, :], qk_ps, w)
                nc.vector.tensor_copy(
                    sc[b][:, c],
                    qk_ps[:].rearrange("p (w r) -> p w r", w=PAGE),
                )
                nc.vector.tensor_reduce(
                    pmaxT[:, c, b, :],
                    qk_ps[:].rearrange("p (w r) -> p r w", w=PAGE),
                    axis=X, op=OP.max,
                )

        # ---------------- suffix attention ----------------
        for b in range(B):
            qrhs = qsb[:, b * R:(b + 1) * R]
            ksuf = kpool.tile([128, NSC, 128], F32, tag="ksuf")
            nc.sync.dma_start(
                out=ksuf[:],
                in_=ck[b, PREF:START, :].rearrange("(w p) d -> p w d", p=128),
            )
            sqk_ps = pp_qk.tile([128, NSC * R], F32, tag="qk")
            for cs in range(NSC):
                qk_block(ksuf[:, cs, :], sqk_ps, cs)
            nc.vector.tensor_copy(
                ssc[b][:], sqk_ps[:].rearrange("p (c r) -> p c r", c=NSC)
            )
            # row max over all suffix tokens
            red = pp_ms.tile([128, 128], F32, tag="ms")
            smax_p = stgpool.tile([128, R], F32, tag="smax")
            nc.vector.tensor_reduce(
                smax_p[:], ssc[b][:].rearrange("p c r -> p r c"),
                axis=X, op=OP.max,
            )
            nc.tensor.transpose(red[:R, :128], smax_p[:], ident[:])
            nc.vector.tensor_reduce(msuf[b][:], red[:R, :128], axis=X, op=OP.max)
            red2 = pp_ms.tile([128, 128], F32, tag="ms")
            nc.tensor.transpose(red2[:1, :R], msuf[b][:], ident[:R, :R])
            nc.vector.tensor_copy(msuf_f[b][:], red2[:1, :R])
            bcast_rows(msufb[b][:], msuf_f[b][:], R)
            # w = exp(scale*(s - m))
            a_in, a_b = bass.broadcast_tensor_aps(
                ssc[b][:], msufb[b][:].rearrange("p (c r) -> p c r", c=1)
            )
            nc.vector.tensor_tensor(ssc[b][:], a_in, a_b, op=OP.subtract)
            nc.scalar.activation(
                ssc[b][:], ssc[b][:], mybir.ActivationFunctionType.Exp,
                scale=SCALE,
            )
            vsuf = vpool.tile([128, NSC, 128], F32, tag="vsuf")
            nc.sync.dma_start(
                out=vsuf[:],
                in_=cv[b, PREF:START, :].rearrange("(w p) d -> p w d", p=128),
            )
            av_ps = pp_av.tile([R, 128], F32, tag="av")
            den_ps = pp_av.tile([R, 1], F32, tag="den")
            for cs in range(NSC):
                nc.tensor.matmul(
                    av_ps[:], ssc[b][:, cs, :], vsuf[:, cs, :],
                    start=(cs == 0), stop=(cs == NSC - 1),
                )
                nc.tensor.matmul(
                    den_ps[:], ssc[b][:, cs, :], ones[:],
                    start=(cs == 0), stop=(cs == NSC - 1),
                )
            stg = stgpool.tile([R, 132], F32, tag="stg")
            nc.vector.tensor_copy(stg[:, :128], av_ps[:])
            nc.vector.tensor_copy(stg[:, 128:129], den_ps[:])
            nc.vector.tensor_copy(stg[:, 129:130], msuf[b][:])
            nc.vector.memset(stg[:, 130:132], 0.0)
            nc.sync.dma_start(out=out[b, 1], in_=stg[:])

        # ---------------- top-k bisection on page maxes ----------------
        gmaxp = stgpool.tile([128, 8], F32, tag="gm")
        nc.vector.tensor_reduce(
            gmaxp[:], pmaxT[:].rearrange("p c b r -> p (b r) c"),
            axis=X, op=OP.max,
        )
        red = pp_ms.tile([128, 128], F32, tag="ms")
        nc.tensor.transpose(red[:8, :128], gmaxp[:], ident[:])
        nc.vector.tensor_copy(tmp128[:], red[:8, :128])
        nc.vector.tensor_reduce(gmax8[:], tmp128[:], axis=X, op=OP.max)
        red2 = pp_ms.tile([128, 128], F32, tag="ms")
        nc.tensor.transpose(red2[:1, :8], gmax8[:], ident[:8, :8])
        nc.vector.tensor_copy(gmaxf[:], red2[:1, :8])
        bcast_rows(gmaxb[:], gmaxf[:], 8)
        for b in range(B):
            redm = pp_ms.tile([128, 128], F32, tag="ms", name="redm")
            nc.tensor.transpose(
                redm[:R, :1], gmaxf[:, b * R:(b + 1) * R], ident[:1, :1]
            )
            nc.vector.tensor_copy(gmaxm[b][:], redm[:R, :1])
        # lo = min - 1 (reduce min), hi = max + 1
        gminp = stgpool.tile([128, 8], F32, tag="gm")
        nc.vector.tensor_reduce(
            gminp[:], pmaxT[:].rearrange("p c b r -> p (b r) c"),
            axis=X, op=OP.min,
        )
        red3 = pp_ms.tile([128, 128], F32, tag="ms")
        nc.tensor.transpose(red3[:8, :128], gminp[:], ident[:])
        nc.vector.tensor_copy(tmp128[:], red3[:8, :128])
        gmin8 = stgpool.tile([8, 1], F32, tag="gmin8")
        nc.vector.tensor_reduce(gmin8[:], tmp128[:], axis=X, op=OP.min)
        red4 = pp_ms.tile([128, 128], F32, tag="ms")
        nc.tensor.transpose(red4[:1, :8], gmin8[:], ident[:8, :8])
        nc.vector.tensor_copy(lo[:], red4[:1, :8])
        nc.vector.tensor_scalar(lo[:], lo[:], 1.0, None, op0=OP.subtract)
        nc.vector.tensor_scalar(hi[:], gmaxf[:], 1.0, None, op0=OP.add)

        for it in range(BISECT_ITERS):
            nc.vector.tensor_tensor(mid[:], lo[:], hi[:], op=OP.add)
            nc.vector.tensor_scalar(mid[:], mid[:], 0.5, None, op0=OP.mult)
            bcast_rows(midb[:], mid[:], 8)
            a_p, a_m = bass.broadcast_tensor_aps(
                pmaxT[:], midb[:].rearrange("p (c b r) -> p c b r", c=1, b=B)
            )
            nc.vector.tensor_tensor(ge01[:], a_p, a_m, op=OP.is_ge)
            cnt_ps = pp_ms.tile([128, 128], F32, tag="ms")
            nc.tensor.matmul(
                cnt_ps[:1, :NCH * B * R], ones[:],
                ge01[:].rearrange("p c b r -> p (c b r)"),
                start=True, stop=True,
            )
            nc.vector.tensor_reduce(
                cnt[:],
                cnt_ps[:1, :NCH * B * R].rearrange(
                    "p (c b r) -> p (b r) c", c=NCH, b=B
                ),
                axis=X, op=OP.add,
            )
            nc.vector.tensor_scalar(sel[:], cnt[:], float(T) - 0.5, None, op0=OP.is_ge)
            nc.vector.tensor_scalar(nsel[:], cnt[:], float(T) - 0.5, None, op0=OP.is_lt)
            nc.vector.tensor_mul(bt1[:], sel[:], mid[:])
            nc.vector.tensor_mul(bt2[:], nsel[:], lo[:])
            nc.vector.tensor_add(lo[:], bt1[:], bt2[:])
            nc.vector.tensor_mul(bt1[:], nsel[:], mid[:])
            nc.vector.tensor_mul(bt2[:], sel[:], hi[:])
            nc.vector.tensor_add(hi[:], bt1[:], bt2[:])
        bcast_rows(midb[:], lo[:], 8)
        a_p, a_t = bass.broadcast_tensor_aps(
            pmaxT[:], midb[:].rearrange("p (c b r) -> p c b r", c=1, b=B)
        )
        nc.vector.tensor_tensor(pm01[:], a_p, a_t, op=OP.is_ge)

        # ---------------- prefix softmax + AV ----------------
        for b in range(B):
            # s - m  (m = global row max, always in selected set)
            a_s, a_m = bass.broadcast_tensor_aps(
                sc[b][:],
                gmaxb[:, b * R:(b + 1) * R].rearrange(
                    "p (c w r) -> p c w r", c=1, w=1
                ),
            )
            nc.vector.tensor_tensor(sc[b][:], a_s, a_m, op=OP.subtract)
            nc.scalar.activation(
                sc[b][:], sc[b][:], mybir.ActivationFunctionType.Exp,
                scale=SCALE,
            )
            a_s2, a_pm = bass.broadcast_tensor_aps(
                sc[b][:], pm01[:, :, b:b + 1, :]
            )
            nc.vector.tensor_tensor(sc[b][:], a_s2, a_pm, op=OP.mult)
            avp_ps = pp_av.tile([R, 128], F32, tag="av")
            denp_ps = pp_av.tile([R, 1], F32, tag="den")
            for c in range(NCH):
                vsb = vpool.tile([128, PAGE, 128], F32, tag="vsb")
                nc.sync.dma_start(
                    out=vsb[:],
                    in_=cv[b, c * 2048:(c + 1) * 2048, :].rearrange(
                        "(p w) d -> p w d", p=128
                    ),
                )
                for w in range(PAGE):
                    nc.tensor.matmul(
                        avp_ps[:], sc[b][:, c, w, :], vsb[:, w, :],
                        start=(c == 0 and w == 0),
                        stop=(c == NCH - 1 and w == PAGE - 1),
                    )
                    nc.tensor.matmul(
                        denp_ps[:], sc[b][:, c, w, :], ones[:],
                        start=(c == 0 and w == 0),
                        stop=(c == NCH - 1 and w == PAGE - 1),
                    )
            stg = stgpool.tile([R, 132], F32, tag="stg")
            nc.vector.tensor_copy(stg[:, :128], avp_ps[:])
            nc.vector.tensor_copy(stg[:, 128:129], denp_ps[:])
            nc.vector.tensor_copy(stg[:, 129:130], gmaxm[b][:])
            nc.vector.memset(stg[:, 130:132], 0.0)
            nc.sync.dma_start(out=out[b, 0], in_=stg[:])

    nc.finalize()
    return nc


def _rope(t, cos, sin):
    t0, t1 = t[..., 0::2], t[..., 1::2]
    re = t0 * cos - t1 * sin
    im = t0 * sin + t1 * cos
    o = np.empty_like(t)
    o[..., 0::2] = re
    o[..., 1::2] = im
    return o


_NC_CACHE = {}


def kernel(x, freqs_cos, freqs_sin, cache_k, cache_v, wq, wk, wv, wo, start_pos):
    x = np.asarray(x, np.float32)
    cache_k = np.asarray(cache_k, np.float32)
    cache_v = np.asarray(cache_v, np.float32)
    xf = x.reshape(B, DIM)
    xq = (xf @ np.asarray(wq, np.float32).T).reshape(B, H, D)
    xk = (xf @ np.asarray(wk, np.float32).T).reshape(B, HKV, D)
    xv = (xf @ np.asarray(wv, np.float32).T).reshape(B, HKV, D)
    cos = np.asarray(freqs_cos, np.float32)[0]
    sin = np.asarray(freqs_sin, np.float32)[0]
    xq = _rope(xq, cos, sin)
    xk = _rope(xk, cos, sin)

    if "nc" not in _NC_CACHE:
        _NC_CACHE["nc"] = build_nc()
    nc = _NC_CACHE["nc"]

    in_maps = []
    for c in range(HKV):
        qh = xq[:, c * R:(c + 1) * R, :]            # [B, R, D]
        in_maps.append({
            "ck": np.ascontiguousarray(cache_k[:, :, c, :]),
            "cv": np.ascontiguousarray(cache_v[:, :, c, :]),
            "qT": np.ascontiguousarray(qh.transpose(2, 0, 1).reshape(D, B * R)),
        })

    trace = bool(int(os.environ.get("KERNEL_TRACE", "0")))
    try:
        res = run_bass_kernel_spmd(
            nc, in_maps, core_ids=list(range(HKV)), trace=trace
        )
        if trace and res.exec_time_ns is not None:
            print(f"HW exec time: {res.exec_time_ns} ns")
    except Exception as e:  # device path unavailable: host fallback
        print(f"kernel: device path failed ({type(e).__name__}); host fallback")
        return _host_reference(x, xq, xk, xv, cache_k, cache_v, wo)

    # host-side merge in float64 for stability
    outacc = np.zeros((B, H, D), np.float64)
    for cidx in range(HKV):
        o = np.asarray(res.results[cidx]["out"], np.float64)  # [B, 2, R, 132]
        qh = np.asarray(xq[:, cidx * R:(cidx + 1) * R, :], np.float64)
        for b in range(B):
            for r in range(R):
                pnum = o[b, 0, r, :128]
                pden = o[b, 0, r, 128]
                pm = SCALE * o[b, 0, r, 129]
                lse_p = pm + np.log(pden)
                out_p = pnum / pden

                snum = o[b, 1, r, :128]
                sden = o[b, 1, r, 128]
                sm = SCALE * o[b, 1, r, 129]
                # fold in the freshly-written token (key/value of this step)
                s_new = SCALE * float(
                    qh[b, r] @ np.asarray(xk[b, cidx], np.float64)
                )
                M = max(sm, s_new)
                wn = np.exp(s_new - M)
                snum = snum * np.exp(sm - M) + wn * np.asarray(xv[b, cidx], np.float64)
                sden = sden * np.exp(sm - M) + wn
                lse_s = M + np.log(sden)
                out_s = snum / sden

                lse = np.logaddexp(lse_p, lse_s)
                outacc[b, cidx * R + r] = (
                    out_p * np.exp(lse_p - lse) + out_s * np.exp(lse_s - lse)
                )

    flat = outacc.reshape(B, H * D).astype(np.float32)
    y = flat @ np.asarray(wo, np.float32).T
    return y.reshape(B, 1, DIM).astype(np.float32)


def _host_reference(x, xq, xk, xv, cache_k, cache_v, wo):
    scale = np.float32(1.0 / np.sqrt(D))
    xqf = xq.reshape(B, 1, H, D).astype(np.float32)
    xkf = xk.reshape(B, 1, HKV, D).astype(np.float32)
    xvf = xv.reshape(B, 1, HKV, D).astype(np.float32)

    def attn(q, k, v):
        s = np.einsum("bqhd,bkhd->bhqk", q, k) * scale
        m = s.max(axis=-1, keepdims=True)
        e = np.exp(s - m)
        den = e.sum(axis=-1, keepdims=True)
        lse = (m + np.log(den))[..., 0]
        o = np.einsum("bhqk,bkhd->bqhd", e / den, v)
        return o, lse

    pref = START - WINDOW
    rep = lambda t: np.repeat(t, R, axis=2)
    k_suf = np.concatenate([cache_k[:, pref:START], xkf], axis=1)
    v_suf = np.concatenate([cache_v[:, pref:START], xvf], axis=1)
    s_out, s_lse = attn(xqf, rep(k_suf), rep(v_suf))

    n_pages = pref // PAGE
    ckp = cache_k[:, :pref].reshape(B, n_pages, PAGE, HKV, D)
    cvp = cache_v[:, :pref].reshape(B, n_pages, PAGE, HKV, D)
    xq_ = xqf.reshape(B, 1, HKV, R, D)
    scores = np.einsum("NSPHD,NLHRD->NSPHR", ckp, xq_).max(axis=2)
    Tn = min(n_pages, TOPK // PAGE)
    top = np.argsort(-scores, axis=1, kind="stable")[:, :Tn]
    idx = np.swapaxes(top, 2, 3).reshape(B, Tn * R, HKV)
    idxb = np.broadcast_to(
        idx[:, :, None, :, None], (B, Tn * R, PAGE, HKV, D)
    )

    def gather(paged):
        g = np.take_along_axis(paged, idxb, axis=1)
        g = g.reshape(B, Tn, R, PAGE, HKV, D).transpose(0, 1, 3, 4, 2, 5)
        return g.reshape(B, Tn * PAGE, H, D)

    p_out, p_lse = attn(xqf, gather(ckp), gather(cvp))
    lse = np.logaddexp(p_lse, s_lse)
    pw = np.exp(p_lse - lse).swapaxes(1, 2)[..., None]
    sw = np.exp(s_lse - lse).swapaxes(1, 2)[..., None]
    o = p_out * pw + s_out * sw
    y = o.reshape(B, 1, H * D).astype(np.float32) @ np.asarray(wo, np.float32).T
    return y.reshape(B, 1, DIM).astype(np.float32)



# revision 8
# speedup vs baseline: 3.5289x; 3.5289x over previous
import os
import sys

import numpy as np

sys.path.insert(0, "/opt/trn_rl_repo")

import concourse.bacc as bacc
import concourse.bass as bass
import concourse.mybir as mybir
from concourse import masks
from concourse.bass_utils import run_bass_kernel_spmd
from concourse.tile import TileContext

B, DIM, H, HKV, D = 2, 4096, 32, 8, 128
R = H // HKV
PAGE, WINDOW, TOPK = 16, 4096, 4096
START = 32768
PREF = START - WINDOW          # 28672 prefix tokens
NCHP = PREF // 2048            # 14 prefix chunks (128 pages x 16 tok)
NCHS = WINDOW // 2048          # 2 suffix chunks
NCH = NCHP + NCHS              # 16 chunks total per batch
T = TOPK // PAGE               # 256 pages selected
SCALE = 1.0 / float(np.sqrt(D))
BISECT_ITERS = 20
F16 = mybir.dt.float16

F32 = mybir.dt.float32
X = mybir.AxisListType.X
OP = mybir.AluOpType


def build_nc():
    nc = bacc.Bacc()
    kT = nc.declare_dram_parameter("kT", [B, NCH, D, 2048], F16, isOutput=False)
    v = nc.declare_dram_parameter("v", [B, START, D], F16, isOutput=False)
    qT = nc.declare_dram_parameter("qT", [D, B * R], F16, isOutput=False)
    out = nc.declare_dram_parameter("out", [B, 2, R, 132], F32, isOutput=True)

    from contextlib import ExitStack

    with TileContext(nc) as tc, ExitStack() as es:
        cpool = es.enter_context(tc.tile_pool(name="consts", bufs=1))
        ident = cpool.tile([128, 128], F32)
        masks.make_identity(nc, ident[:])
        ones = cpool.tile([128, 1], F32)
        nc.vector.memset(ones[:], 1.0)
        allones = cpool.tile([128, 128], F32)
        nc.vector.memset(allones[:], 1.0)
        qsb = cpool.tile([128, B * R], F16)
        nc.sync.dma_start(out=qsb[:], in_=qT[:, :])
        ones_row = cpool.tile([1, 128], F32)
        nc.vector.memset(ones_row[:], 1.0)

        spool = es.enter_context(tc.tile_pool(name="state", bufs=1))
        # raw scores, page-swizzled: [part=page, (chunk, within, head)]
        sc = [spool.tile([128, NCH, PAGE, R], F32, name=f"sc{i}") for i in range(B)]
        # exp'd weights in fp16, same layout
        wp = [spool.tile([128, NCH, PAGE, R], F16, name=f"wp{i}") for i in range(B)]
        # per-chunk denominator partials [128, NCH, R] per batch
        dall = [spool.tile([128, NCH, R], F32, name=f"dall{i}") for i in range(B)]
        pmaxT = spool.tile([128, NCHP, B, R], F32)   # per-page max scores
        pm01 = spool.tile([128, NCHP, B, R], F16)    # selection mask (0/1)
        ge01 = spool.tile([128, NCHP, B, R], F32)
        gmax8 = spool.tile([8, 1], F32)
        gmaxf = spool.tile([1, 8], F32)
        lo1 = spool.tile([1, 8], F32)
        hi1 = spool.tile([1, 8], F32)
        lo128 = spool.tile([128, 8], F32)
        hi128 = spool.tile([128, 8], F32)
        mid128 = spool.tile([128, 8], F32)
        cnt128 = spool.tile([128, 8], F32)
        cmp128 = spool.tile([128, 8], F32)
        d1 = spool.tile([128, 8], F32)
        d2 = spool.tile([128, 8], F32)
        gmaxm = [spool.tile([R, 1], F32, name=f"gmaxm{i}") for i in range(B)]
        tmp128 = spool.tile([8, 128], F32)
        msuf = [spool.tile([R, 1], F32, name=f"msuf{i}") for i in range(B)]
        msuf_f = [spool.tile([1, R], F32, name=f"msuff{i}") for i in range(B)]
        msufb = [spool.tile([128, R], F32, name=f"msufb{i}") for i in range(B)]
        gmaxb = spool.tile([128, 8], F32)

        kpool = es.enter_context(tc.tile_pool(name="k", bufs=4))
        vspool = es.enter_context(tc.tile_pool(name="vs", bufs=4))
        vppool = es.enter_context(tc.tile_pool(name="vp", bufs=28))
        stgpool = es.enter_context(tc.tile_pool(name="stg", bufs=2))

        pp_qk = es.enter_context(tc.tile_pool(name="pp_qk", bufs=3, space="PSUM"))
        pp_av = es.enter_context(tc.tile_pool(name="pp_av", bufs=2, space="PSUM"))
        pp_ms = es.enter_context(tc.tile_pool(name="pp_ms", bufs=2, space="PSUM"))

        def bcast_rows(dst_sb, src_1xn, n):
            # replicate [1, n] across 128 partitions via PE outer product
            bc_ps = pp_ms.tile([128, 128], F32, tag="ms", name="bc_ps")
            nc.tensor.matmul(bc_ps[:, :n], ones_row[:], src_1xn,
                             start=True, stop=True)
            nc.vector.tensor_copy(dst_sb, bc_ps[:, :n])

        # ---------------- QK over all chunks (suffix chunks first) ---------
        # K DMAs only in this phase so the K stream gets full HBM bandwidth.
        ch_order = [(b, ch) for b in range(B)
                    for ch in list(range(NCHP, NCH)) + list(range(NCHP))]
        for b, ch in ch_order:
            ksb = kpool.tile([128, 2048], F16, tag="ksb")
            nc.sync.dma_start(out=ksb[:], in_=kT[b, ch, :, :])
            qk_ps = pp_qk.tile([128, PAGE * R], F32, tag="qk")
            qrhs = qsb[:, b * R:(b + 1) * R]
            for w in range(PAGE):
                nc.tensor.matmul(
                    qk_ps[:, w * R:(w + 1) * R],
                    ksb[:, w * 128:(w + 1) * 128], qrhs,
                    start=True, stop=True,
                )
            nc.vector.tensor_copy(
                sc[b][:, ch],
                qk_ps[:].rearrange("p (w r) -> p w r", w=PAGE),
            )
            if ch < NCHP:
                nc.vector.tensor_reduce(
                    pmaxT[:, ch, b, :],
                    qk_ps[:].rearrange("p (w r) -> p r w", w=PAGE),
                    axis=X, op=OP.max,
                )

        # ---------------- V DMAs (issued after all K, so K streams first) ---
        vsuf = {}
        for b in range(B):
            for cs in range(NCHS):
                t = vspool.tile([128, PAGE, 128], F16, tag="vsuf")
                ch = NCHP + cs
                nc.sync.dma_start(
                    out=t[:],
                    in_=v[b, ch * 2048:(ch + 1) * 2048, :].rearrange(
                        "(p w) d -> p w d", p=128
                    ),
                )
                vsuf[(b, cs)] = t
        vpre = {}
        for b in range(B):
            for c in range(NCHP):
                t = vppool.tile([128, PAGE, 128], F16, tag="vpre")
                nc.sync.dma_start(
                    out=t[:],
                    in_=v[b, c * 2048:(c + 1) * 2048, :].rearrange(
                        "(p w) d -> p w d", p=128
                    ),
                )
                vpre[(b, c)] = t

        # ---------------- suffix attention ----------------
        for b in range(B):
            smax_p = stgpool.tile([128, R], F32, tag="smax")
            nc.vector.tensor_reduce(
                smax_p[:],
                sc[b][:, NCHP:NCH].rearrange("p c w r -> p r (c w)"),
                axis=X, op=OP.max,
            )
            red = pp_ms.tile([128, 128], F32, tag="ms")
            nc.tensor.transpose(red[:R, :128], smax_p[:], ident[:])
            nc.vector.tensor_reduce(msuf[b][:], red[:R, :128], axis=X, op=OP.max)
            red2 = pp_ms.tile([128, 128], F32, tag="ms")
            nc.tensor.transpose(red2[:1, :R], msuf[b][:], ident[:R, :R])
            nc.vector.tensor_copy(msuf_f[b][:], red2[:1, :R])
            bcast_rows(msufb[b][:], msuf_f[b][:], R)
            a_in, a_b = bass.broadcast_tensor_aps(
                sc[b][:, NCHP:NCH],
                msufb[b][:].rearrange("p (c w r) -> p c w r", c=1, w=1),
            )
            nc.vector.tensor_tensor(sc[b][:, NCHP:NCH], a_in, a_b, op=OP.subtract)
            nc.scalar.activation(
                wp[b][:, NCHP:NCH], sc[b][:, NCHP:NCH],
                mybir.ActivationFunctionType.Exp, scale=SCALE,
            )
            for cs in range(NCHS):
                nc.vector.tensor_reduce(
                    dall[b][:, NCHP + cs, :],
                    wp[b][:, NCHP + cs].rearrange("p w r -> p r w"),
                    axis=X, op=OP.add,
                )
            av_ps = pp_av.tile([R, 128], F32, tag="av")
            for cs in range(NCHS):
                vt = vsuf[(b, cs)]
                for w in range(PAGE):
                    nc.tensor.matmul(
                        av_ps[:], wp[b][:, NCHP + cs, w, :], vt[:, w, :],
                        start=(cs == 0 and w == 0),
                        stop=(cs == NCHS - 1 and w == PAGE - 1),
                    )
            dred = stgpool.tile([128, R], F32, tag="dred")
            nc.vector.tensor_reduce(
                dred[:], dall[b][:, NCHP:NCH].rearrange("p c r -> p r c"),
                axis=X, op=OP.add,
            )
            d_ps = pp_ms.tile([128, 128], F32, tag="ms")
            nc.tensor.matmul(d_ps[:1, :R], ones[:], dred[:], start=True, stop=True)
            d_sb = stgpool.tile([1, R], F32, tag="dsb")
            nc.vector.tensor_copy(d_sb[:], d_ps[:1, :R])
            dT_ps = pp_ms.tile([128, 128], F32, tag="ms")
            nc.tensor.transpose(dT_ps[:R, :1], d_sb[:], ident[:1, :1])
            stg = stgpool.tile([R, 132], F32, tag="stg")
            nc.vector.tensor_copy(stg[:, :128], av_ps[:])
            nc.vector.tensor_copy(stg[:, 128:129], dT_ps[:R, :1])
            nc.vector.tensor_copy(stg[:, 129:130], msuf[b][:])
            nc.vector.memset(stg[:, 130:132], 0.0)
            nc.sync.dma_start(out=out[b, 1], in_=stg[:])

        # ---------------- global max/min + bisection bounds ----------------
        gmaxp = stgpool.tile([128, 8], F32, tag="gm")
        nc.vector.tensor_reduce(
            gmaxp[:], pmaxT[:].rearrange("p c b r -> p (b r) c"),
            axis=X, op=OP.max,
        )
        red = pp_ms.tile([128, 128], F32, tag="ms")
        nc.tensor.transpose(red[:8, :128], gmaxp[:], ident[:])
        nc.vector.tensor_copy(tmp128[:], red[:8, :128])
        nc.vector.tensor_reduce(gmax8[:], tmp128[:], axis=X, op=OP.max)
        red2 = pp_ms.tile([128, 128], F32, tag="ms")
        nc.tensor.transpose(red2[:1, :8], gmax8[:], ident[:8, :8])
        nc.vector.tensor_copy(gmaxf[:], red2[:1, :8])
        bcast_rows(gmaxb[:], gmaxf[:], 8)
        for b in range(B):
            redm = pp_ms.tile([128, 128], F32, tag="ms", name="redm")
            nc.tensor.transpose(
                redm[:R, :1], gmaxf[:, b * R:(b + 1) * R], ident[:1, :1]
            )
            nc.vector.tensor_copy(gmaxm[b][:], redm[:R, :1])
        gminp = stgpool.tile([128, 8], F32, tag="gm")
        nc.vector.tensor_reduce(
            gminp[:], pmaxT[:].rearrange("p c b r -> p (b r) c"),
            axis=X, op=OP.min,
        )
        red3 = pp_ms.tile([128, 128], F32, tag="ms")
        nc.tensor.transpose(red3[:8, :128], gminp[:], ident[:])
        nc.vector.tensor_copy(tmp128[:], red3[:8, :128])
        gmin8 = stgpool.tile([8, 1], F32, tag="gmin8")
        nc.vector.tensor_reduce(gmin8[:], tmp128[:], axis=X, op=OP.min)
        red4 = pp_ms.tile([128, 128], F32, tag="ms")
        nc.tensor.transpose(red4[:1, :8], gmin8[:], ident[:8, :8])
        nc.vector.tensor_copy(lo1[:], red4[:1, :8])
        nc.vector.tensor_scalar(lo1[:], lo1[:], 1.0, None, op0=OP.subtract)
        nc.vector.tensor_scalar(hi1[:], gmaxf[:], 1.0, None, op0=OP.add)
        bcast_rows(lo128[:], lo1[:], 8)
        bcast_rows(hi128[:], hi1[:], 8)

        # ------ unmasked exp of prefix scores (ACT work overlaps bisection) -
        for b in range(B):
            a_s, a_m = bass.broadcast_tensor_aps(
                sc[b][:, :NCHP],
                gmaxb[:, b * R:(b + 1) * R].rearrange(
                    "p (c w r) -> p c w r", c=1, w=1
                ),
            )
            nc.vector.tensor_tensor(sc[b][:, :NCHP], a_s, a_m, op=OP.subtract)
            nc.scalar.activation(
                wp[b][:, :NCHP], sc[b][:, :NCHP],
                mybir.ActivationFunctionType.Exp, scale=SCALE,
            )

        # ------- top-k bisection (counts replicated via all-ones matmul) ----
        for it in range(BISECT_ITERS):
            nc.vector.tensor_add(mid128[:], lo128[:], hi128[:])
            nc.vector.tensor_scalar(mid128[:], mid128[:], 0.5, None, op0=OP.mult)
            a_p, a_m = bass.broadcast_tensor_aps(
                pmaxT[:], mid128[:].rearrange("p (c b r) -> p c b r", c=1, b=B)
            )
            nc.vector.tensor_tensor(ge01[:], a_p, a_m, op=OP.is_ge)
            cnt_ps = pp_ms.tile([128, 128], F32, tag="ms")
            nc.tensor.matmul(
                cnt_ps[:, :NCHP * B * R], allones[:],
                ge01[:].rearrange("p c b r -> p (c b r)"),
                start=True, stop=True,
            )
            nc.vector.tensor_reduce(
                cnt128[:],
                cnt_ps[:, :NCHP * B * R].rearrange(
                    "p (c b r) -> p (b r) c", c=NCHP, b=B
                ),
                axis=X, op=OP.add,
            )
            nc.vector.tensor_scalar(
                cmp128[:], cnt128[:], float(T) - 0.5, None, op0=OP.is_ge
            )
            # cmp=1 -> lo=mid ; cmp=0 -> hi=mid
            nc.vector.tensor_sub(d1[:], mid128[:], lo128[:])
            nc.vector.tensor_mul(d1[:], d1[:], cmp128[:])
            nc.vector.tensor_add(lo128[:], lo128[:], d1[:])
            nc.vector.tensor_sub(d2[:], hi128[:], mid128[:])
            nc.vector.tensor_mul(d2[:], d2[:], cmp128[:])
            nc.vector.tensor_add(hi128[:], mid128[:], d2[:])
        a_p, a_t = bass.broadcast_tensor_aps(
            pmaxT[:], lo128[:].rearrange("p (c b r) -> p c b r", c=1, b=B)
        )
        nc.vector.tensor_tensor(pm01[:], a_p, a_t, op=OP.is_ge)

        # ---------------- prefix mask + AV ----------------
        for b in range(B):
            a_w, a_pm = bass.broadcast_tensor_aps(
                wp[b][:, :NCHP], pm01[:, :, b:b + 1, :]
            )
            nc.vector.tensor_tensor(wp[b][:, :NCHP], a_w, a_pm, op=OP.mult)
            for c in range(NCHP):
                nc.vector.tensor_reduce(
                    dall[b][:, c, :],
                    wp[b][:, c].rearrange("p w r -> p r w"),
                    axis=X, op=OP.add,
                )
            avp_ps = pp_av.tile([R, 128], F32, tag="av")
            for c in range(NCHP):
                vt = vpre[(b, c)]
                for w in range(PAGE):
                    nc.tensor.matmul(
                        avp_ps[:], wp[b][:, c, w, :], vt[:, w, :],
                        start=(c == 0 and w == 0),
                        stop=(c == NCHP - 1 and w == PAGE - 1),
                    )
            dred = stgpool.tile([128, R], F32, tag="dred")
            nc.vector.tensor_reduce(
                dred[:], dall[b][:, :NCHP].rearrange("p c r -> p r c"),
                axis=X, op=OP.add,
            )
            d_ps = pp_ms.tile([128, 128], F32, tag="ms")
            nc.tensor.matmul(d_ps[:1, :R], ones[:], dred[:], start=True, stop=True)
            d_sb = stgpool.tile([1, R], F32, tag="dsb")
            nc.vector.tensor_copy(d_sb[:], d_ps[:1, :R])
            dT_ps = pp_ms.tile([128, 128], F32, tag="ms")
            nc.tensor.transpose(dT_ps[:R, :1], d_sb[:], ident[:1, :1])
            stg = stgpool.tile([R, 132], F32, tag="stg")
            nc.vector.tensor_copy(stg[:, :128], avp_ps[:])
            nc.vector.tensor_copy(stg[:, 128:129], dT_ps[:R, :1])
            nc.vector.tensor_copy(stg[:, 129:130], gmaxm[b][:])
            nc.vector.memset(stg[:, 130:132], 0.0)
            nc.sync.dma_start(out=out[b, 0], in_=stg[:])

    nc.finalize()
    return nc


def _rope(t, cos, sin):
    t0, t1 = t[..., 0::2], t[..., 1::2]
    re = t0 * cos - t1 * sin
    im = t0 * sin + t1 * cos
    o = np.empty_like(t)
    o[..., 0::2] = re
    o[..., 1::2] = im
    return o


_NC_CACHE = {}


def kernel(x, freqs_cos, freqs_sin, cache_k, cache_v, wq, wk, wv, wo, start_pos):
    x = np.asarray(x, np.float32)
    cache_k = np.asarray(cache_k, np.float32)
    cache_v = np.asarray(cache_v, np.float32)
    xf = x.reshape(B, DIM)
    xq = (xf @ np.asarray(wq, np.float32).T).reshape(B, H, D)
    xk = (xf @ np.asarray(wk, np.float32).T).reshape(B, HKV, D)
    xv = (xf @ np.asarray(wv, np.float32).T).reshape(B, HKV, D)
    cos = np.asarray(freqs_cos, np.float32)[0]
    sin = np.asarray(freqs_sin, np.float32)[0]
    xq = _rope(xq, cos, sin)
    xk = _rope(xk, cos, sin)

    if "nc" not in _NC_CACHE:
        _NC_CACHE["nc"] = build_nc()
    nc = _NC_CACHE["nc"]

    in_maps = []
    for c in range(HKV):
        qh = xq[:, c * R:(c + 1) * R, :]            # [B, R, D]
        kc = cache_k[:, :, c, :]                     # [B, 32768, 128]
        # swizzle: kT[b, ch, d, w*128+p] = K[b, ch*2048 + p*16 + w, d]
        kt = np.ascontiguousarray(
            kc.reshape(B, NCH, 128, PAGE, D).transpose(0, 1, 4, 3, 2)
        ).astype(np.float16)
        in_maps.append({
            "kT": kt,
            "v": np.ascontiguousarray(cache_v[:, :, c, :]).astype(np.float16),
            "qT": np.ascontiguousarray(
                qh.transpose(2, 0, 1).reshape(D, B * R)
            ).astype(np.float16),
        })

    trace = bool(int(os.environ.get("KERNEL_TRACE", "0")))
    try:
        res = run_bass_kernel_spmd(
            nc, in_maps, core_ids=list(range(HKV)), trace=trace
        )
        if trace and res.exec_time_ns is not None:
            print(f"HW exec time: {res.exec_time_ns} ns")
    except Exception as e:  # device path unavailable: host fallback
        print(f"kernel: device path failed ({type(e).__name__}); host fallback")
        return _host_reference(x, xq, xk, xv, cache_k, cache_v, wo)

    # host-side merge in float64 for stability
    outacc = np.zeros((B, H, D), np.float64)
    for cidx in range(HKV):
        o = np.asarray(res.results[cidx]["out"], np.float64)  # [B, 2, R, 132]
        qh = np.asarray(xq[:, cidx * R:(cidx + 1) * R, :], np.float64)
        for b in range(B):
            for r in range(R):
                pnum = o[b, 0, r, :128]
                pden = o[b, 0, r, 128]
                pm = SCALE * o[b, 0, r, 129]
                lse_p = pm + np.log(pden)
                out_p = pnum / pden

                snum = o[b, 1, r, :128]
                sden = o[b, 1, r, 128]
                sm = SCALE * o[b, 1, r, 129]
                # fold in the freshly-written token (key/value of this step)
                s_new = SCALE * float(
                    qh[b, r] @ np.asarray(xk[b, cidx], np.float64)
                )
                M = max(sm, s_new)
                wn = np.exp(s_new - M)
                snum = snum * np.exp(sm - M) + wn * np.asarray(xv[b, cidx], np.float64)
                sden = sden * np.exp(sm - M) + wn
                lse_s = M + np.log(sden)
                out_s = snum / sden

                lse = np.logaddexp(lse_p, lse_s)
                outacc[b, cidx * R + r] = (
                    out_p * np.exp(lse_p - lse) + out_s * np.exp(lse_s - lse)
                )

    flat = outacc.reshape(B, H * D).astype(np.float32)
    y = flat @ np.asarray(wo, np.float32).T
    return y.reshape(B, 1, DIM).astype(np.float32)


def _host_reference(x, xq, xk, xv, cache_k, cache_v, wo):
    scale = np.float32(1.0 / np.sqrt(D))
    xqf = xq.reshape(B, 1, H, D).astype(np.float32)
    xkf = xk.reshape(B, 1, HKV, D).astype(np.float32)
    xvf = xv.reshape(B, 1, HKV, D).astype(np.float32)

    def attn(q, k, v):
        s = np.einsum("bqhd,bkhd->bhqk", q, k) * scale
        m = s.max(axis=-1, keepdims=True)
        e = np.exp(s - m)
        den = e.sum(axis=-1, keepdims=True)
        lse = (m + np.log(den))[..., 0]
        o = np.einsum("bhqk,bkhd->bqhd", e / den, v)
        return o, lse

    pref = START - WINDOW
    rep = lambda t: np.repeat(t, R, axis=2)
    k_suf = np.concatenate([cache_k[:, pref:START], xkf], axis=1)
    v_suf = np.concatenate([cache_v[:, pref:START], xvf], axis=1)
    s_out, s_lse = attn(xqf, rep(k_suf), rep(v_suf))

    n_pages = pref // PAGE
    ckp = cache_k[:, :pref].reshape(B, n_pages, PAGE, HKV, D)
    cvp = cache_v[:, :pref].reshape(B, n_pages, PAGE, HKV, D)
    xq_ = xqf.reshape(B, 1, HKV, R, D)
    scores = np.einsum("NSPHD,NLHRD->NSPHR", ckp, xq_).max(axis=2)
    Tn = min(n_pages, TOPK // PAGE)
    top = np.argsort(-scores, axis=1, kind="stable")[:, :Tn]
    idx = np.swapaxes(top, 2, 3).reshape(B, Tn * R, HKV)
    idxb = np.broadcast_to(
        idx[:, :, None, :, None], (B, Tn * R, PAGE, HKV, D)
    )

    def gather(paged):
        g = np.take_along_axis(paged, idxb, axis=1)
        g = g.reshape(B, Tn, R, PAGE, HKV, D).transpose(0, 1, 3, 4, 2, 5)
        return g.reshape(B, Tn * PAGE, H, D)

    p_out, p_lse = attn(xqf, gather(ckp), gather(cvp))
    lse = np.logaddexp(p_lse, s_lse)
    pw = np.exp(p_lse - lse).swapaxes(1, 2)[..., None]
    sw = np.exp(s_lse - lse).swapaxes(1, 2)[..., None]
    o = p_out * pw + s_out * sw
    y = o.reshape(B, 1, H * D).astype(np.float32) @ np.asarray(wo, np.float32).T
    return y.reshape(B, 1, DIM).astype(np.float32)
